# revision 12
# baseline (speedup 1.0000x reference)
"""Trainium2 Bass kernel for BistableModeBank (gnn_message_passing).

out = X + dt * (mu_z*X - X^3 + a_z*deg_z*X + b_z*(W_z @ X))

Strategy: pure data-parallel over 8 cores. Per core:
  out = X*(1 - dt*x^2)  (fp32, natural layout)  +  D_z @ X  (fp16 matmuls,
  transposed layout), where D_m = dt*(diag(mu_m + a_m*deg_m) + b_m*W_m).
The per-item mode select (z in {0..3}) telescopes over thresholds:
  D_z = E_0 + [z>=1]E_1 + [z>=2]E_2 + [z>=3]E_3,  E_k = D_k - D_{k-1}
so D_z@x = E_0@x + E_1@(s1*x) + E_2@(s2*s1*x) + E_3@(s3*s2*s1*x) folds into
four PSUM-accumulating matmuls over exactly-masked fp16 inputs.

Transposed layout (from a single xbar DMA-transpose per tile):
  xt[4g+j, 128b+c] = X[item = S0/4 + 512c + 32b + g, feature j]
"""

import sys

sys.path.insert(0, "/opt/trn_rl_repo")

import numpy as np

import concourse.bacc as bacc
import concourse.mybir as mybir
from concourse.tile import TileContext
from concourse.bass_utils import run_bass_kernel_spmd

N_CORES = 8
B_FULL = 4_194_304
NF = 4  # features per item
MU_MIN, MU_MAX = 0.1, 1.5
PER_CORE = B_FULL // N_CORES  # 524288 items
F = 512  # matmul free dim (one PSUM bank)
SUPER = 4  # groups per super-group
C = F * SUPER  # 2048: super-group width in floats per partition
SUPER_ITEMS = 128 * C // 4  # 65536 items per super-group
N_SUPER = PER_CORE // SUPER_ITEMS  # 8

_PROGRAM_CACHE: dict = {}

f32 = mybir.dt.float32
f16 = mybir.dt.float16
i32 = mybir.dt.int32
Alu = mybir.AluOpType


def _build_program(n_super: int):
    nc = bacc.Bacc("TRN2", target_bir_lowering=False)

    X = nc.dram_tensor("X", (n_super, 128, C), f32, kind="ExternalInput")
    Z = nc.dram_tensor("Z", (n_super, 128, C // 4), i32, kind="ExternalInput")
    L_dram = [
        nc.dram_tensor(f"L{k}", (128, 128), f16, kind="ExternalInput")
        for k in range(4)
    ]
    R_dram = [
        nc.dram_tensor(f"R{u}", (128, 128), f16, kind="ExternalInput")
        for u in range(4)
    ]
    NEGDT = nc.dram_tensor("NEGDT", (128, 1), f32, kind="ExternalInput")
    OUT = nc.dram_tensor("OUT", (n_super, 128, C), f32, kind="ExternalOutput")

    with TileContext(nc) as tc:
        with (
            tc.tile_pool(name="wpool", bufs=1) as wpool,
            tc.tile_pool(name="work", bufs=2) as pool,
            tc.tile_pool(name="zps", bufs=2, space="PSUM") as zps_pool,
            tc.tile_pool(name="yps", bufs=2, space="PSUM") as yps_pool,
        ):
            # Load constants once.
            Ls = []
            for k in range(4):
                t = wpool.tile([128, 128], f16, tag=f"L{k}")
                nc.sync.dma_start(out=t, in_=L_dram[k][:, :])
                Ls.append(t)
            Rs = []
            for u in range(4):
                t = wpool.tile([128, 128], f16, tag=f"R{u}")
                nc.sync.dma_start(out=t, in_=R_dram[u][:, :])
                Rs.append(t)
            negdt = wpool.tile([128, 1], f32, tag="negdt")
            nc.sync.dma_start(out=negdt, in_=NEGDT[:, :])

            for sg in range(n_super):
                # ---- loads (natural layout, contiguous 1 MB / 256 KB) ----
                xn = pool.tile([128, C], f32, tag="xn")
                nc.sync.dma_start(out=xn, in_=X[sg])
                # z: cast int32 -> fp16 during the SWDGE load
                zf = pool.tile([128, C // 4], f16, tag="zf")
                nc.gpsimd.dma_start(out=zf, in_=Z[sg])

                # ---- fp16 cast of X (ACT) ----
                xf = pool.tile([128, C], f16, tag="xf")
                nc.scalar.copy(out=xf, in_=xn)

                # ---- single-instruction block transposes ----
                xt = pool.tile([128, C], f16, tag="xt")
                nc.sync.dma_start(
                    out=xt.rearrange("p (u c) -> p u c", u=C // 128),
                    in_=xf, transpose=True,
                )
                zt = pool.tile([128, C // 4], f16, tag="zt")
                nc.sync.dma_start(
                    out=zt.rearrange("p (u c) -> p u c", u=C // 512),
                    in_=zf, transpose=True,
                )

                # ---- broadcast z across the 4 feature rows via PE ----
                zrep_list = []
                for q in range(SUPER):
                    zrep_ps = zps_pool.tile([128, F], f32, tag="zrep_ps")
                    for r in range(4):
                        nc.tensor.matmul(
                            out=zrep_ps[:, r * 128:(r + 1) * 128],
                            lhsT=Rs[r],
                            rhs=zt[:, q * 128:(q + 1) * 128],
                            start=True,
                            stop=True,
                        )
                    zrep_list.append(zrep_ps)
                zrep = pool.tile([128, C], f16, tag="zrep")
                for q in range(SUPER):
                    if q % 2 == 0:
                        nc.vector.tensor_copy(
                            out=zrep[:, q * F:(q + 1) * F], in_=zrep_list[q]
                        )
                    else:
                        nc.scalar.copy(
                            out=zrep[:, q * F:(q + 1) * F], in_=zrep_list[q]
                        )

                # ---- threshold masks of z: m_k = [z >= k] ----
                m1 = pool.tile([128, C], f16, tag="m1")
                nc.vector.tensor_scalar(
                    out=m1, in0=zrep, scalar1=0.5, scalar2=None, op0=Alu.is_ge
                )
                m2 = pool.tile([128, C], f16, tag="m2")
                nc.vector.tensor_scalar(
                    out=m2, in0=zrep, scalar1=1.5, scalar2=None, op0=Alu.is_ge
                )
                m3 = pool.tile([128, C], f16, tag="m3")
                nc.vector.tensor_scalar(
                    out=m3, in0=zrep, scalar1=2.5, scalar2=None, op0=Alu.is_ge
                )

                # ---- telescoped masked inputs (exact fp16 products) ----
                u1 = pool.tile([128, C], f16, tag="u1")
                nc.vector.tensor_mul(out=u1, in0=xt, in1=m1)
                u2 = pool.tile([128, C], f16, tag="u2")
                nc.vector.tensor_mul(out=u2, in0=u1, in1=m2)
                u3 = pool.tile([128, C], f16, tag="u3")
                nc.vector.tensor_mul(out=u3, in0=u2, in1=m3)

                # ---- mode-blended coupling: 4 accumulating matmuls / group ----
                dT = pool.tile([128, C], f16, tag="dT")
                for q in range(SUPER):
                    sl = slice(q * F, (q + 1) * F)
                    y_ps = yps_pool.tile([128, F], f32, tag="y_ps")
                    nc.tensor.matmul(out=y_ps, lhsT=Ls[0], rhs=xt[:, sl],
                                     start=True, stop=False)
                    nc.tensor.matmul(out=y_ps, lhsT=Ls[1], rhs=u1[:, sl],
                                     start=False, stop=False)
                    nc.tensor.matmul(out=y_ps, lhsT=Ls[2], rhs=u2[:, sl],
                                     start=False, stop=False)
                    nc.tensor.matmul(out=y_ps, lhsT=Ls[3], rhs=u3[:, sl],
                                     start=False, stop=True)
                    nc.scalar.copy(out=dT[:, sl], in_=y_ps)

                # ---- back to natural layout (fp16 delta) ----
                dN = pool.tile([128, C], f16, tag="dN")
                nc.sync.dma_start(
                    out=dN.rearrange("p (u c) -> p u c", u=C // 128),
                    in_=dT, transpose=True,
                )

                # ---- fp32 cubic path: w = X * (1 - dt*x^2) ----
                x2 = pool.tile([128, C], f32, tag="x2")
                nc.scalar.activation(
                    out=x2, in_=xn, func=mybir.ActivationFunctionType.Square
                )
                s = pool.tile([128, C], f32, tag="s")
                nc.vector.tensor_scalar(
                    out=s, in0=x2, scalar1=negdt[:, 0:1], scalar2=1.0,
                    op0=Alu.mult, op1=Alu.add,
                )
                w = pool.tile([128, C], f32, tag="w")
                nc.vector.tensor_mul(out=w, in0=xn, in1=s)

                # ---- final combine via SWDGE accumulate-DMA + store ----
                nc.gpsimd.dma_start(out=w, in_=dN, accum_op=Alu.add)
                nc.sync.dma_start(out=OUT[sg], in_=w)

    nc.compile()
    return nc


def _mode_matrices(graph_logits, mu_logits, alpha_param, beta_param, dt):
    """Host-side tiny parameter preprocessing -> D_m (4x4 per mode)."""
    gl = np.asarray(graph_logits, dtype=np.float64)
    ml = np.asarray(mu_logits, dtype=np.float64)
    al = np.asarray(alpha_param, dtype=np.float64)
    be = np.asarray(beta_param, dtype=np.float64)

    S = 0.5 * (gl + np.swapaxes(gl, 1, 2))
    W = 1.0 / (1.0 + np.exp(-S)) * (1.0 - np.eye(NF))  # (M,N,N)
    mus = MU_MIN + (MU_MAX - MU_MIN) / (1.0 + np.exp(-ml))  # (M,N)
    deg = W.sum(axis=-1)  # (M,N)
    D = np.zeros((4, NF, NF))
    for m in range(4):
        D[m] = dt * (np.diag(mus[m] + al[m] * deg[m]) + be[m] * W[m])
    return D


def _weight_tensors(D):
    """Telescoped threshold expansion -> blockdiag lhsT tiles (fp16)."""
    E = [
        D[0],
        D[1] - D[0],
        D[2] - D[1],
        D[3] - D[2],
    ]
    Ls = [
        np.kron(np.eye(32), Ek.T).astype(np.float16) for Ek in E
    ]  # lhsT[4g+j, 4g+i] = E[i, j]
    Rs = []
    for u in range(4):
        R = np.zeros((128, 128), dtype=np.float16)
        for g in range(32):
            for j in range(4):
                R[32 * u + g, 4 * g + j] = 1.0
        Rs.append(R)
    return Ls, Rs


def kernel(**inputs) -> np.ndarray:
    X = np.ascontiguousarray(np.asarray(inputs["X"], dtype=np.float32))
    z = np.asarray(inputs["z"])
    dt = float(np.asarray(inputs["dt_val"]).reshape(-1)[0])
    D = _mode_matrices(
        inputs["graph_logits"], inputs["mu_logits"],
        inputs["alpha_param"], inputs["beta_param"], dt,
    )
    Lmats, Rmats = _weight_tensors(D)

    B = X.shape[0]
    assert B == B_FULL, f"expected {B_FULL} items, got {B}"
    z32 = np.ascontiguousarray(z.astype(np.int32))

    if "prog" not in _PROGRAM_CACHE:
        _PROGRAM_CACHE["prog"] = _build_program(N_SUPER)
    nc = _PROGRAM_CACHE["prog"]

    negdt = np.full((128, 1), -dt, dtype=np.float32)
    in_maps = []
    for c in range(N_CORES):
        xs = X[c * PER_CORE:(c + 1) * PER_CORE].reshape(N_SUPER, 128, C)
        zs = z32[c * PER_CORE:(c + 1) * PER_CORE].reshape(N_SUPER, 128, C // 4)
        m = {"X": xs, "Z": zs, "NEGDT": negdt}
        for k in range(4):
            m[f"L{k}"] = Lmats[k]
            m[f"R{k}"] = Rmats[k]
        in_maps.append(m)

    res = run_bass_kernel_spmd(
        nc, in_maps, core_ids=list(range(N_CORES)),
        **_PROGRAM_CACHE.get("run_kwargs", {})
    )
    _PROGRAM_CACHE["last_result"] = res
    out = np.empty((B_FULL, NF), dtype=np.float32)
    for c in range(N_CORES):
        out[c * PER_CORE:(c + 1) * PER_CORE] = (
            res.results[c]["OUT"].reshape(PER_CORE, NF)
        )
    return out


# revision 16
# speedup vs baseline: 1.0816x; 1.0816x over previous
"""Trainium2 Bass kernel for BistableModeBank (gnn_message_passing).

out = X + dt * (mu_z*X - X^3 + a_z*deg_z*X + b_z*(W_z @ X))

Strategy: pure data-parallel over 8 cores. Per core:
  out = X*(1 - dt*x^2)  (fp32, natural layout)  +  D_z @ X  (fp16 matmuls,
  transposed layout), where D_m = dt*(diag(mu_m + a_m*deg_m) + b_m*W_m).
The per-item mode select (z in {0..3}) telescopes over thresholds:
  D_z = E_0 + [z>=1]E_1 + [z>=2]E_2 + [z>=3]E_3,  E_k = D_k - D_{k-1}
so D_z@x = E_0@x + E_1@(s1*x) + E_2@(s2*s1*x) + E_3@(s3*s2*s1*x) folds into
four PSUM-accumulating matmuls over exactly-masked fp16 inputs.

Transposed layout (from a single xbar DMA-transpose per tile):
  xt[4g+j, 128b+c] = X[item = S0/4 + 512c + 32b + g, feature j]
"""

import sys

sys.path.insert(0, "/opt/trn_rl_repo")

import numpy as np

import concourse.bacc as bacc
import concourse.mybir as mybir
from concourse.tile import TileContext
from concourse.bass_utils import run_bass_kernel_spmd

N_CORES = 8
B_FULL = 4_194_304
NF = 4  # features per item
MU_MIN, MU_MAX = 0.1, 1.5
PER_CORE = B_FULL // N_CORES  # 524288 items
F = 512  # matmul free dim (one PSUM bank)
SUPER = 4  # groups per super-group
C = F * SUPER  # 2048: super-group width in floats per partition
SUPER_ITEMS = 128 * C // 4  # 65536 items per super-group
N_SUPER = PER_CORE // SUPER_ITEMS  # 8

_PROGRAM_CACHE: dict = {}

f32 = mybir.dt.float32
f16 = mybir.dt.float16
i32 = mybir.dt.int32
Alu = mybir.AluOpType


def _build_program(n_super: int):
    nc = bacc.Bacc("TRN2", target_bir_lowering=False)

    X = nc.dram_tensor("X", (n_super, 128, C), f32, kind="ExternalInput")
    Z = nc.dram_tensor("Z", (n_super, 128, C // 4), i32, kind="ExternalInput")
    L_dram = [
        nc.dram_tensor(f"L{k}", (128, 128), f16, kind="ExternalInput")
        for k in range(4)
    ]
    R_dram = [
        nc.dram_tensor(f"R{u}", (128, 128), f16, kind="ExternalInput")
        for u in range(4)
    ]
    NEGDT = nc.dram_tensor("NEGDT", (128, 1), f32, kind="ExternalInput")
    OUT = nc.dram_tensor("OUT", (n_super, 128, C), f32, kind="ExternalOutput")

    with TileContext(nc) as tc:
        with (
            tc.tile_pool(name="wpool", bufs=1) as wpool,
            tc.tile_pool(name="work", bufs=3) as pool,
            tc.tile_pool(name="zps", bufs=2, space="PSUM") as zps_pool,
            tc.tile_pool(name="yps", bufs=6, space="PSUM") as yps_pool,
        ):
            # Load constants once.
            Ls = []
            for k in range(4):
                t = wpool.tile([128, 128], f16, tag=f"L{k}")
                nc.sync.dma_start(out=t, in_=L_dram[k][:, :])
                Ls.append(t)
            Rs = []
            for u in range(4):
                t = wpool.tile([128, 128], f16, tag=f"R{u}")
                nc.sync.dma_start(out=t, in_=R_dram[u][:, :])
                Rs.append(t)
            negdt = wpool.tile([128, 1], f32, tag="negdt")
            nc.sync.dma_start(out=negdt, in_=NEGDT[:, :])

            for sg in range(n_super):
                # ---- loads (natural layout, contiguous 1 MB / 256 KB) ----
                xn = pool.tile([128, C], f32, tag="xn")
                nc.sync.dma_start(out=xn, in_=X[sg])
                # z: cast int32 -> fp16 during the SWDGE load
                zf = pool.tile([128, C // 4], f16, tag="zf")
                nc.gpsimd.dma_start(out=zf, in_=Z[sg])

                # ---- fp16 cast of X (ACT) ----
                xf = pool.tile([128, C], f16, tag="xf")
                nc.scalar.copy(out=xf, in_=xn)

                # ---- single-instruction block transposes ----
                xt = pool.tile([128, C], f16, tag="xt")
                nc.sync.dma_start(
                    out=xt.rearrange("p (u c) -> p u c", u=C // 128),
                    in_=xf, transpose=True,
                )
                zt = pool.tile([128, C // 4], f16, tag="zt")
                nc.sync.dma_start(
                    out=zt.rearrange("p (u c) -> p u c", u=C // 512),
                    in_=zf, transpose=True,
                )

                # ---- broadcast z across the 4 feature rows via PE ----
                zrep_list = []
                for q in range(SUPER):
                    zrep_ps = zps_pool.tile([128, F], f32, tag="zrep_ps")
                    for r in range(4):
                        nc.tensor.matmul(
                            out=zrep_ps[:, r * 128:(r + 1) * 128],
                            lhsT=Rs[r],
                            rhs=zt[:, q * 128:(q + 1) * 128],
                            start=True,
                            stop=True,
                        )
                    zrep_list.append(zrep_ps)
                zrep = pool.tile([128, C], f16, tag="zrep")
                for q in range(SUPER):
                    nc.scalar.copy(
                        out=zrep[:, q * F:(q + 1) * F], in_=zrep_list[q]
                    )

                # ---- threshold masks of z: m_k = [z >= k] ----
                m1 = pool.tile([128, C], f16, tag="m1")
                nc.vector.tensor_scalar(
                    out=m1, in0=zrep, scalar1=0.5, scalar2=None, op0=Alu.is_ge
                )
                m2 = pool.tile([128, C], f16, tag="m2")
                nc.vector.tensor_scalar(
                    out=m2, in0=zrep, scalar1=1.5, scalar2=None, op0=Alu.is_ge
                )
                m3 = pool.tile([128, C], f16, tag="m3")
                nc.vector.tensor_scalar(
                    out=m3, in0=zrep, scalar1=2.5, scalar2=None, op0=Alu.is_ge
                )

                # ---- telescoped masked inputs (exact fp16 products) ----
                # One u tile, rewritten in place between matmul rounds:
                #   round 0: rhs = xt          (weights E_0)
                #   round k: u *= m_k; rhs = u (weights E_k)
                yps_list = []
                for q in range(SUPER):
                    y_ps = yps_pool.tile([128, F], f32, tag="y_ps")
                    nc.tensor.matmul(out=y_ps, lhsT=Ls[0],
                                     rhs=xt[:, q * F:(q + 1) * F],
                                     start=True, stop=False)
                    yps_list.append(y_ps)
                u = pool.tile([128, C], f16, tag="u")
                nc.vector.tensor_mul(out=u, in0=xt, in1=m1)
                for k, mk in ((1, None), (2, m2), (3, m3)):
                    if mk is not None:
                        nc.vector.tensor_mul(out=u, in0=u, in1=mk)
                    for q in range(SUPER):
                        nc.tensor.matmul(out=yps_list[q], lhsT=Ls[k],
                                         rhs=u[:, q * F:(q + 1) * F],
                                         start=False, stop=(k == 3))
                dT = pool.tile([128, C], f16, tag="dT")
                for q in range(SUPER):
                    nc.scalar.copy(out=dT[:, q * F:(q + 1) * F], in_=yps_list[q])

                # ---- back to natural layout (fp16 delta) ----
                dN = pool.tile([128, C], f16, tag="dN")
                nc.sync.dma_start(
                    out=dN.rearrange("p (u c) -> p u c", u=C // 128),
                    in_=dT, transpose=True,
                )

                # ---- fp32 cubic path: w = X * (1 - dt*x^2) ----
                x2 = pool.tile([128, C], f32, tag="x2")
                nc.scalar.activation(
                    out=x2, in_=xn, func=mybir.ActivationFunctionType.Square
                )
                # s = 1 - dt*x^2, in place over x2
                nc.vector.tensor_scalar(
                    out=x2, in0=x2, scalar1=negdt[:, 0:1], scalar2=1.0,
                    op0=Alu.mult, op1=Alu.add,
                )
                # w = x * s, in place over xn; then o = w + delta, in place
                nc.vector.tensor_mul(out=xn, in0=xn, in1=x2)
                nc.vector.tensor_add(out=xn, in0=xn, in1=dN)
                nc.sync.dma_start(out=OUT[sg], in_=xn)

    nc.compile()
    return nc


def _mode_matrices(graph_logits, mu_logits, alpha_param, beta_param, dt):
    """Host-side tiny parameter preprocessing -> D_m (4x4 per mode)."""
    gl = np.asarray(graph_logits, dtype=np.float64)
    ml = np.asarray(mu_logits, dtype=np.float64)
    al = np.asarray(alpha_param, dtype=np.float64)
    be = np.asarray(beta_param, dtype=np.float64)

    S = 0.5 * (gl + np.swapaxes(gl, 1, 2))
    W = 1.0 / (1.0 + np.exp(-S)) * (1.0 - np.eye(NF))  # (M,N,N)
    mus = MU_MIN + (MU_MAX - MU_MIN) / (1.0 + np.exp(-ml))  # (M,N)
    deg = W.sum(axis=-1)  # (M,N)
    D = np.zeros((4, NF, NF))
    for m in range(4):
        D[m] = dt * (np.diag(mus[m] + al[m] * deg[m]) + be[m] * W[m])
    return D


def _weight_tensors(D):
    """Telescoped threshold expansion -> blockdiag lhsT tiles (fp16)."""
    E = [
        D[0],
        D[1] - D[0],
        D[2] - D[1],
        D[3] - D[2],
    ]
    Ls = [
        np.kron(np.eye(32), Ek.T).astype(np.float16) for Ek in E
    ]  # lhsT[4g+j, 4g+i] = E[i, j]
    Rs = []
    for u in range(4):
        R = np.zeros((128, 128), dtype=np.float16)
        for g in range(32):
            for j in range(4):
                R[32 * u + g, 4 * g + j] = 1.0
        Rs.append(R)
    return Ls, Rs


def kernel(**inputs) -> np.ndarray:
    X = np.ascontiguousarray(np.asarray(inputs["X"], dtype=np.float32))
    z = np.asarray(inputs["z"])
    dt = float(np.asarray(inputs["dt_val"]).reshape(-1)[0])
    D = _mode_matrices(
        inputs["graph_logits"], inputs["mu_logits"],
        inputs["alpha_param"], inputs["beta_param"], dt,
    )
    Lmats, Rmats = _weight_tensors(D)

    B = X.shape[0]
    assert B == B_FULL, f"expected {B_FULL} items, got {B}"
    z32 = np.ascontiguousarray(z.astype(np.int32))

    if "prog" not in _PROGRAM_CACHE:
        _PROGRAM_CACHE["prog"] = _build_program(N_SUPER)
    nc = _PROGRAM_CACHE["prog"]

    negdt = np.full((128, 1), -dt, dtype=np.float32)
    in_maps = []
    for c in range(N_CORES):
        xs = X[c * PER_CORE:(c + 1) * PER_CORE].reshape(N_SUPER, 128, C)
        zs = z32[c * PER_CORE:(c + 1) * PER_CORE].reshape(N_SUPER, 128, C // 4)
        m = {"X": xs, "Z": zs, "NEGDT": negdt}
        for k in range(4):
            m[f"L{k}"] = Lmats[k]
            m[f"R{k}"] = Rmats[k]
        in_maps.append(m)

    res = run_bass_kernel_spmd(
        nc, in_maps, core_ids=list(range(N_CORES)),
        **_PROGRAM_CACHE.get("run_kwargs", {})
    )
    _PROGRAM_CACHE["last_result"] = res
    out = np.empty((B_FULL, NF), dtype=np.float32)
    for c in range(N_CORES):
        out[c * PER_CORE:(c + 1) * PER_CORE] = (
            res.results[c]["OUT"].reshape(PER_CORE, NF)
        )
    return out
